# revision 13
# baseline (speedup 1.0000x reference)
"""Trainium2 Bass kernel for nn_AttentionMap (B=4, S=4096, D=256 full attention).

Sharding: 8 cores = 4 batches x 2 query-halves (data-parallel batch,
sequence-parallel query rows). No collectives: each core computes
out[b, h*2048:(h+1)*2048, :] from conv_local[b] and its conv_global slice.

Per-core algorithm (all matmuls contract over the partition dim):
  phase 0: load X=conv_local[b] [4096,256], G=conv_global slice [2048,256];
           PE-transpose into XT [256,4096], GT [256,2048] (d on partitions).
  phase 1: KT = Wk^T XT + bk  [256,4096]   (lhsT=Wk chunk, rhs=XT chunk)
           QT = Wq^T GT + bq  [256,2048]
           V  = X Wv + bv     [4096,257]   (lhsT=XT chunk, rhs=Wv chunk),
           with an appended ones-column (V[:,256]=1) so the PV matmul also
           produces the softmax denominator.
  phase 2: per q-tile of 512 query rows:
           S^T chunks [128s,512q] = KT_chunk^T @ QT_tile (PSUM, fp32 accum)
           expS = exp(S^T / sqrt(256))  (ACT, PSUM->SBUF)   [no max-sub;
             scores ~ N(0,1) so exp is safe in fp32]
           O_unnorm[128q,257] = sum_s expS_chunk^T @ V_chunk  (PSUM accum);
           col 256 = sum_s exp = softmax denominator.
           out = O_unnorm[:, :256] * reciprocal(O_unnorm[:, 256])  -> DMA out.

MM_MODE selects the PE operand dtype: "f32" (exact, 4 cyc/row),
"f32r" (1 cyc/row, fp32 storage, operands rounded by producers),
"bf16" (1 cyc/row, half SBUF). PSUM accumulation is fp32 in all modes.
"""

import os
import sys
from contextlib import ExitStack

import numpy as np

for _p in ("/opt/trn_rl_repo",):
    if _p not in sys.path and os.path.isdir(_p):
        sys.path.insert(0, _p)

import concourse.bass as bass
import concourse.mybir as mybir
import concourse.tile as tile
from concourse import bacc
from concourse.bass_utils import run_bass_kernel_spmd
from concourse.masks import make_identity

B = 4
S = 4096          # kv sequence length (= full query length)
D = 256           # model dim = head dim
NCORES = 8
SQH = S // 2      # query rows per core (2048)
QT = 512          # query tile (moving free dim of the S^T matmuls)
NQT = SQH // QT   # 4
NSC = S // 128    # 32 kv chunks of 128
NDC = D // 128    # 2 d chunks of 128
VPAD = 2          # ones-columns appended to V (even free dim for f32r matmul)
F32 = mybir.dt.float32
F32R = mybir.dt.float32r
BF16 = mybir.dt.bfloat16

# "f32" (exact), "f32r" (fast fp32 PE mode), "bf16"
MM_MODE = os.environ.get("ATTN_MM_MODE", "f32")

_CACHED = {}


def build_program():
    nc = bacc.Bacc("TRN2", target_bir_lowering=False, debug=False)

    x_d = nc.dram_tensor("x", [S, D], F32, kind="ExternalInput").ap()
    g_d = nc.dram_tensor("g", [SQH, D], F32, kind="ExternalInput").ap()
    wk_d = nc.dram_tensor("wk", [D, D], F32, kind="ExternalInput").ap()
    wq_d = nc.dram_tensor("wq", [D, D], F32, kind="ExternalInput").ap()
    wv_d = nc.dram_tensor("wv", [D, D], F32, kind="ExternalInput").ap()
    bk_d = nc.dram_tensor("bk", [D, 1], F32, kind="ExternalInput").ap()
    bq_d = nc.dram_tensor("bq", [D, 1], F32, kind="ExternalInput").ap()
    bv_d = nc.dram_tensor("bv", [1, D], F32, kind="ExternalInput").ap()
    out_d = nc.dram_tensor("out", [SQH, D], F32, kind="ExternalOutput").ap()

    lowp = MM_MODE in ("bf16", "f32r")
    sb_dt = {"f32": F32, "f32r": F32R, "bf16": BF16}[MM_MODE]
    # dtype of the transpose datapath (input tiles + psum out must match)
    tr_dt = BF16 if MM_MODE == "bf16" else F32

    with tile.TileContext(nc) as tc, ExitStack() as ctx:
        Copy = mybir.ActivationFunctionType.Copy
        Ident = mybir.ActivationFunctionType.Identity
        Exp = mybir.ActivationFunctionType.Exp

        consts = ctx.enter_context(tc.tile_pool(name="consts", bufs=1))
        big = ctx.enter_context(tc.tile_pool(name="big", bufs=1))

        ident = consts.tile([128, 128], tr_dt)
        make_identity(nc, ident[:])

        wk_sb = consts.tile([128, NDC, D], sb_dt)
        wq_sb = consts.tile([128, NDC, D], sb_dt)
        wv_sb = consts.tile([128, NDC, D], sb_dt)
        bk_sb = consts.tile([128, NDC, 1], F32)
        bq_sb = consts.tile([128, NDC, 1], F32)
        ones1 = consts.tile([1, 128], sb_dt)
        ones1_f32 = consts.tile([1, 128], F32)
        vone_f32 = consts.tile([128, NSC, VPAD], F32)
        bv_bc = consts.tile([128, D], F32)

        if lowp:
            wld = consts.tile([128, 3 * NDC, D], F32, tag="wld")
            for kc in range(NDC):
                nc.sync.dma_start(wld[:, 0 * NDC + kc, :], wk_d[kc * 128:(kc + 1) * 128, :])
                nc.sync.dma_start(wld[:, 1 * NDC + kc, :], wq_d[kc * 128:(kc + 1) * 128, :])
                nc.sync.dma_start(wld[:, 2 * NDC + kc, :], wv_d[kc * 128:(kc + 1) * 128, :])
            for kc in range(NDC):
                nc.vector.tensor_copy(wk_sb[:, kc, :], wld[:, 0 * NDC + kc, :])
                nc.vector.tensor_copy(wq_sb[:, kc, :], wld[:, 1 * NDC + kc, :])
                nc.vector.tensor_copy(wv_sb[:, kc, :], wld[:, 2 * NDC + kc, :])
            bv_ld = consts.tile([1, D], F32, tag="bvl")
            nc.sync.dma_start(bv_ld[:], bv_d[:])
            bv_rhs = consts.tile([1, D], sb_dt, tag="bvc")
            nc.vector.tensor_copy(bv_rhs[:], bv_ld[:])
        else:
            for kc in range(NDC):
                nc.sync.dma_start(wk_sb[:, kc, :], wk_d[kc * 128:(kc + 1) * 128, :])
                nc.sync.dma_start(wq_sb[:, kc, :], wq_d[kc * 128:(kc + 1) * 128, :])
                nc.sync.dma_start(wv_sb[:, kc, :], wv_d[kc * 128:(kc + 1) * 128, :])
            bv_rhs = consts.tile([1, D], F32, tag="bvc")
            nc.sync.dma_start(bv_rhs[:], bv_d[:])
        for kc in range(NDC):
            nc.sync.dma_start(bk_sb[:, kc, :], bk_d[kc * 128:(kc + 1) * 128, :])
            nc.sync.dma_start(bq_sb[:, kc, :], bq_d[kc * 128:(kc + 1) * 128, :])
        # memset on a float32r tile is invalid ISA; stage through f32 + copy
        nc.vector.memset(ones1_f32[:], 1.0)
        nc.vector.tensor_copy(ones1[:], ones1_f32[:])
        nc.vector.memset(vone_f32[:], 1.0)

        # ---- phase 2 SBUF residents (allocated first so they survive) ----
        kt = big.tile([128, NDC, S], sb_dt)       # K^T  [d, s]
        qt_sb = big.tile([128, NDC, SQH], sb_dt)  # Q^T  [d, q]
        vt = big.tile([128, NSC, D + VPAD], sb_dt)  # V||1 [s, d+pad]

        with ExitStack() as p01:
            ld = p01.enter_context(tc.tile_pool(name="ld", bufs=4))
            trp = p01.enter_context(tc.tile_pool(name="trp", bufs=3, space="PSUM"))
            xtgt = p01.enter_context(tc.tile_pool(name="xtgt", bufs=1))
            mmp = p01.enter_context(tc.tile_pool(name="mmp", bufs=3, space="PSUM"))

            # bv broadcast across partitions via a K=1 matmul
            psb = mmp.tile([128, D], F32, tag="proj")
            nc.tensor.matmul(psb[:], ones1[:], bv_rhs[:], start=True, stop=True)
            nc.vector.tensor_copy(bv_bc[:], psb[:])

            xt = xtgt.tile([128, NDC, S], sb_dt)    # X^T [d, s]
            gt = xtgt.tile([128, NDC, SQH], sb_dt)  # G^T [d, q]

            # ---- phase 0: load + transpose X and G ----
            for t in range(NSC):
                xld = ld.tile([128, D], F32, tag="ld")
                nc.sync.dma_start(xld[:], x_d[t * 128:(t + 1) * 128, :])
                if MM_MODE == "bf16":
                    xldc = ld.tile([128, D], BF16, tag="ldc")
                    nc.vector.tensor_copy(xldc[:], xld[:])
                    xsrc = xldc
                else:
                    xsrc = xld
                for kc in range(NDC):
                    ps = trp.tile([128, 128], tr_dt, tag="tr")
                    nc.tensor.transpose(ps[:], xsrc[:, kc * 128:(kc + 1) * 128], ident[:])
                    if (t + kc) % 2 == 0:
                        nc.scalar.activation(xt[:, kc, t * 128:(t + 1) * 128], ps[:], Copy)
                    else:
                        nc.vector.tensor_copy(xt[:, kc, t * 128:(t + 1) * 128], ps[:])
            for t in range(SQH // 128):
                gld = ld.tile([128, D], F32, tag="ld")
                nc.sync.dma_start(gld[:], g_d[t * 128:(t + 1) * 128, :])
                if MM_MODE == "bf16":
                    gldc = ld.tile([128, D], BF16, tag="ldc")
                    nc.vector.tensor_copy(gldc[:], gld[:])
                    gsrc = gldc
                else:
                    gsrc = gld
                for kc in range(NDC):
                    ps = trp.tile([128, 128], tr_dt, tag="tr")
                    nc.tensor.transpose(ps[:], gsrc[:, kc * 128:(kc + 1) * 128], ident[:])
                    if (t + kc) % 2 == 0:
                        nc.scalar.activation(gt[:, kc, t * 128:(t + 1) * 128], ps[:], Copy)
                    else:
                        nc.vector.tensor_copy(gt[:, kc, t * 128:(t + 1) * 128], ps[:])

            # ---- phase 1: projections ----
            # KT[dc, s] = sum_kc Wk[kc,dc]^T @ XT[kc, s] + bk[dc]
            for dc in range(NDC):
                for nt in range(S // 512):
                    ps = mmp.tile([128, 512], F32, tag="proj")
                    for kc in range(NDC):
                        nc.tensor.matmul(
                            ps[:],
                            wk_sb[:, kc, dc * 128:(dc + 1) * 128],
                            xt[:, kc, nt * 512:(nt + 1) * 512],
                            start=(kc == 0), stop=(kc == NDC - 1),
                        )
                    nc.vector.tensor_scalar_add(kt[:, dc, nt * 512:(nt + 1) * 512],
                                                ps[:], bk_sb[:, dc, :])
            for dc in range(NDC):
                for nt in range(SQH // 512):
                    ps = mmp.tile([128, 512], F32, tag="proj")
                    for kc in range(NDC):
                        nc.tensor.matmul(
                            ps[:],
                            wq_sb[:, kc, dc * 128:(dc + 1) * 128],
                            gt[:, kc, nt * 512:(nt + 1) * 512],
                            start=(kc == 0), stop=(kc == NDC - 1),
                        )
                    nc.vector.tensor_scalar_add(qt_sb[:, dc, nt * 512:(nt + 1) * 512],
                                                ps[:], bq_sb[:, dc, :])
            # V[s, :256] = X @ Wv + bv ; V[s, 256] = 1
            for t in range(NSC):
                ps = mmp.tile([128, D], F32, tag="proj")
                for kc in range(NDC):
                    nc.tensor.matmul(
                        ps[:],
                        xt[:, kc, t * 128:(t + 1) * 128],
                        wv_sb[:, kc, :],
                        start=(kc == 0), stop=(kc == NDC - 1),
                    )
                nc.vector.tensor_add(vt[:, t, 0:D], ps[:], bv_bc[:])
            nc.vector.tensor_copy(vt[:, :, D:D + VPAD], vone_f32[:])

        # ---- phase 2: attention ----
        es_bufs = 2 if MM_MODE == "bf16" else 1
        esp = ctx.enter_context(tc.tile_pool(name="esp", bufs=es_bufs))
        # each stp tile spans 2 PSUM banks so one ACTIVATE handles 2 kv-chunks
        stp = ctx.enter_context(tc.tile_pool(name="stp", bufs=2, space="PSUM"))
        pvp = ctx.enter_context(tc.tile_pool(name="pvp", bufs=1, space="PSUM"))
        osb_p = ctx.enter_context(tc.tile_pool(name="osb", bufs=4))

        inv_sqrt_d = 1.0 / float(np.sqrt(D))
        nqs = QT // 128
        for qi in range(NQT):
            q0 = qi * QT
            es = esp.tile([128, NSC, QT], sb_dt, tag="es")
            for tp in range(NSC // 2):
                ps = stp.tile([128, 2 * QT], F32, tag="st")
                for sub in range(2):
                    t = 2 * tp + sub
                    for kc in range(NDC):
                        nc.tensor.matmul(
                            ps[:, sub * QT:(sub + 1) * QT],
                            kt[:, kc, t * 128:(t + 1) * 128],
                            qt_sb[:, kc, q0:q0 + QT],
                            start=(kc == 0), stop=(kc == NDC - 1),
                        )
                nc.scalar.activation(es[:, 2 * tp:2 * tp + 2, :], ps[:], Exp,
                                     scale=inv_sqrt_d)
            accs = []
            for qs in range(nqs):
                acc_t = pvp.tile([128, D + VPAD], F32, tag=f"acc{qs}", name=f"acc{qs}")
                accs.append(acc_t)
            for t in range(NSC):
                for qs in range(nqs):
                    nc.tensor.matmul(
                        accs[qs][:],
                        es[:, t, qs * 128:(qs + 1) * 128],
                        vt[:, t, :],
                        start=(t == 0), stop=(t == NSC - 1),
                    )
            for qs in range(nqs):
                acc = accs[qs]
                osb = osb_p.tile([128, D], F32, tag="osb")
                rec = osb_p.tile([128, 1], F32, tag="rec")
                nc.vector.reciprocal(rec[:], acc[:, D:D + 1])
                nc.vector.tensor_scalar_mul(osb[:], acc[:, 0:D], rec[:])
                nc.sync.dma_start(
                    out_d[q0 + qs * 128:q0 + (qs + 1) * 128, :], osb[:]
                )

    nc.compile()
    return nc


def _get_program():
    if "nc" not in _CACHED:
        _CACHED["nc"] = build_program()
    return _CACHED["nc"]


def kernel(conv_local, conv_global, Wk, bk, Wq, bq, Wv, bv):
    nc = _get_program()
    conv_local = np.ascontiguousarray(np.asarray(conv_local, dtype=np.float32))
    conv_global = np.ascontiguousarray(np.asarray(conv_global, dtype=np.float32))
    wk = np.ascontiguousarray(np.asarray(Wk, dtype=np.float32))
    wq = np.ascontiguousarray(np.asarray(Wq, dtype=np.float32))
    wv = np.ascontiguousarray(np.asarray(Wv, dtype=np.float32))
    bk = np.ascontiguousarray(np.asarray(bk, dtype=np.float32).reshape(D, 1))
    bq = np.ascontiguousarray(np.asarray(bq, dtype=np.float32).reshape(D, 1))
    bv = np.ascontiguousarray(np.asarray(bv, dtype=np.float32).reshape(1, D))

    in_maps = []
    for c in range(NCORES):
        b, h = c // 2, c % 2
        in_maps.append({
            "x": conv_local[b],
            "g": np.ascontiguousarray(conv_global[b, h * SQH:(h + 1) * SQH]),
            "wk": wk, "wq": wq, "wv": wv,
            "bk": bk, "bq": bq, "bv": bv,
        })

    trace = bool(int(os.environ.get("ATTN_TRACE", "0")))
    res = run_bass_kernel_spmd(nc, in_maps, list(range(NCORES)), trace=trace)
    _CACHED["last_results"] = res

    out = np.empty((B, S, D), dtype=np.float32)
    for c in range(NCORES):
        b, h = c // 2, c % 2
        out[b, h * SQH:(h + 1) * SQH] = res.results[c]["out"]
    return out


# revision 30
# speedup vs baseline: 1.5291x; 1.5291x over previous
"""Trainium2 Bass kernel for nn_AttentionMap (B=4, S=4096, D=256 full attention).

Sharding: 8 cores = 4 batches x 2 query-halves (data-parallel batch,
sequence-parallel query rows). No collectives: each core computes
out[b, h*2048:(h+1)*2048, :] from conv_local[b] and its conv_global slice.

Per-core algorithm (all matmuls contract over the partition dim):
  phase 0: load X=conv_local[b] [4096,256], G=conv_global slice [2048,256];
           PE-transpose into XT [256,4096], GT [256,2048] (d on partitions).
  phase 1: KT = Wk^T XT + bk  [256,4096]   (lhsT=Wk chunk, rhs=XT chunk)
           QT = Wq^T GT + bq  [256,2048]
           V  = X Wv + bv     [4096,257]   (lhsT=XT chunk, rhs=Wv chunk),
           with an appended ones-column (V[:,256]=1) so the PV matmul also
           produces the softmax denominator.
  phase 2: per q-tile of 512 query rows:
           S^T chunks [128s,512q] = KT_chunk^T @ QT_tile (PSUM, fp32 accum)
           expS = exp(S^T / sqrt(256))  (ACT, PSUM->SBUF)   [no max-sub;
             scores ~ N(0,1) so exp is safe in fp32]
           O_unnorm[128q,257] = sum_s expS_chunk^T @ V_chunk  (PSUM accum);
           col 256 = sum_s exp = softmax denominator.
           out = O_unnorm[:, :256] * reciprocal(O_unnorm[:, 256])  -> DMA out.

MM_MODE selects the PE operand dtype: "f32" (exact, 4 cyc/row),
"f32r" (1 cyc/row, fp32 storage, operands rounded by producers),
"bf16" (1 cyc/row, half SBUF). PSUM accumulation is fp32 in all modes.
"""

import os
import sys
from contextlib import ExitStack

import numpy as np

for _p in ("/opt/trn_rl_repo",):
    if _p not in sys.path and os.path.isdir(_p):
        sys.path.insert(0, _p)

import concourse.bass as bass
import concourse.mybir as mybir
import concourse.tile as tile
from concourse import bacc
from concourse.bass_utils import run_bass_kernel_spmd
from concourse.masks import make_identity

B = 4
S = 4096          # kv sequence length (= full query length)
D = 256           # model dim = head dim
NCORES = 8
SQH = S // 2      # query rows per core (2048)
QT = 512          # query tile (moving free dim of the S^T matmuls)
NQT = SQH // QT   # 4
NSC = S // 128    # 32 kv chunks of 128
NDC = D // 128    # 2 d chunks of 128
VPAD = 2          # ones-columns appended to V (even free dim for f32r matmul)
F32 = mybir.dt.float32
F32R = mybir.dt.float32r
BF16 = mybir.dt.bfloat16
F16 = mybir.dt.float16

# "f32" (exact), "f32r" (fast fp32 PE mode), "bf16"
MM_MODE = os.environ.get("ATTN_MM_MODE", "f32")
ES_SPLIT = bool(int(os.environ.get("ATTN_ES_SPLIT", "1")))

_CACHED = {}


def build_program(bench_reps: int = 0):
    """bench_reps > 0 wraps phase 2 in a hardware For_i loop (timing only)."""
    nc = bacc.Bacc("TRN2", target_bir_lowering=False, debug=False)

    x_d = nc.dram_tensor("x", [S, D], F32, kind="ExternalInput").ap()
    g_d = nc.dram_tensor("g", [SQH, D], F32, kind="ExternalInput").ap()
    wk_d = nc.dram_tensor("wk", [D, D], F32, kind="ExternalInput").ap()
    wq_d = nc.dram_tensor("wq", [D, D], F32, kind="ExternalInput").ap()
    wv_d = nc.dram_tensor("wv", [D, D], F32, kind="ExternalInput").ap()
    bq_d = nc.dram_tensor("bq", [D, 1], F32, kind="ExternalInput").ap()
    bv_d = nc.dram_tensor("bv", [1, D], F32, kind="ExternalInput").ap()
    out_d = nc.dram_tensor("out", [SQH, D], F32, kind="ExternalOutput").ap()

    lowp = MM_MODE in ("bf16", "f16", "f32r")
    cast2b = MM_MODE in ("bf16", "f16")  # 2-byte modes: cast inputs pre-transpose
    sb_dt = {"f32": F32, "f32r": F32R, "bf16": BF16, "f16": F16}[MM_MODE]
    # dtype of the transpose datapath (input tiles + psum out must match)
    tr_dt = sb_dt if cast2b else F32

    with tile.TileContext(nc) as tc, ExitStack() as ctx:
        Copy = mybir.ActivationFunctionType.Copy
        Ident = mybir.ActivationFunctionType.Identity
        Exp = mybir.ActivationFunctionType.Exp

        consts = ctx.enter_context(tc.tile_pool(name="consts", bufs=1))
        big = ctx.enter_context(tc.tile_pool(name="big", bufs=1))

        ident = consts.tile([128, 128], tr_dt)
        make_identity(nc, ident[:])

        wk_sb = consts.tile([128, NDC, D], sb_dt)
        wq_sb = consts.tile([128, NDC, D], sb_dt)
        wv_sb = consts.tile([128, NDC, D], sb_dt)
        bq_sb = consts.tile([128, NDC, 1], F32)
        ones1 = consts.tile([1, 128], sb_dt)
        ones1_f32 = consts.tile([1, 128], F32)
        vone_f32 = consts.tile([128, NSC, VPAD], F32)
        bv_bc = consts.tile([128, D], F32)

        if lowp:
            wld = consts.tile([128, 3 * NDC, D], F32, tag="wld")
            for kc in range(NDC):
                nc.sync.dma_start(wld[:, 0 * NDC + kc, :], wk_d[kc * 128:(kc + 1) * 128, :])
                nc.sync.dma_start(wld[:, 1 * NDC + kc, :], wq_d[kc * 128:(kc + 1) * 128, :])
                nc.sync.dma_start(wld[:, 2 * NDC + kc, :], wv_d[kc * 128:(kc + 1) * 128, :])
            for kc in range(NDC):
                if cast2b:
                    nc.vector.tensor_copy(wk_sb[:, kc, :], wld[:, 0 * NDC + kc, :])
                nc.vector.tensor_copy(wq_sb[:, kc, :], wld[:, 1 * NDC + kc, :])
                nc.vector.tensor_copy(wv_sb[:, kc, :], wld[:, 2 * NDC + kc, :])
            bv_ld = consts.tile([1, D], F32, tag="bvl")
            nc.sync.dma_start(bv_ld[:], bv_d[:])
            bv_rhs = consts.tile([1, D], sb_dt, tag="bvc")
            nc.vector.tensor_copy(bv_rhs[:], bv_ld[:])
        else:
            for kc in range(NDC):
                nc.sync.dma_start(wk_sb[:, kc, :], wk_d[kc * 128:(kc + 1) * 128, :])
                nc.sync.dma_start(wq_sb[:, kc, :], wq_d[kc * 128:(kc + 1) * 128, :])
                nc.sync.dma_start(wv_sb[:, kc, :], wv_d[kc * 128:(kc + 1) * 128, :])
            bv_rhs = consts.tile([1, D], F32, tag="bvc")
            nc.sync.dma_start(bv_rhs[:], bv_d[:])
        for kc in range(NDC):
            nc.sync.dma_start(bq_sb[:, kc, :], bq_d[kc * 128:(kc + 1) * 128, :])
        # memset on a float32r tile is invalid ISA; stage through f32 + copy
        nc.vector.memset(ones1_f32[:], 1.0)
        nc.vector.tensor_copy(ones1[:], ones1_f32[:])
        nc.vector.memset(vone_f32[:], 1.0)

        # ---- phase 2 SBUF residents (allocated first so they survive) ----
        # scores^T = XT.T @ YT where YT = Wk^T @ QhatT: the bk bias only adds
        # a per-query-row constant to scores, which softmax cancels exactly,
        # so K never needs to be materialized at all.
        xt = big.tile([128, NDC, S], sb_dt)       # X^T [d, s]
        yt = big.tile([128, NDC, SQH], sb_dt)     # Wk^T Qhat^T [d, q]
        vt = big.tile([128, NSC, D + VPAD], sb_dt)  # V||1 [s, d+pad]

        with ExitStack() as p01:
            ld = p01.enter_context(tc.tile_pool(name="ld", bufs=8))
            trp = p01.enter_context(tc.tile_pool(name="trp", bufs=3, space="PSUM"))
            xtgt = p01.enter_context(tc.tile_pool(name="xtgt", bufs=1))
            mmp = p01.enter_context(tc.tile_pool(name="mmp", bufs=3, space="PSUM"))

            # bv broadcast across partitions via a K=1 matmul
            psb = mmp.tile([128, D], F32, tag="proj")
            nc.tensor.matmul(psb[:], ones1[:], bv_rhs[:], start=True, stop=True)
            nc.vector.tensor_copy(bv_bc[:], psb[:])

            gt = xtgt.tile([128, NDC, SQH], sb_dt)   # G^T [d, q]
            qt_sb = xtgt.tile([128, NDC, SQH], sb_dt)  # Qhat^T [d, q] (temp)

            # Wk^T chunks for the YT projection: wkT[:, a, b*128:] = Wk[b,a]^T
            wkT_sb = consts.tile([128, NDC, D], sb_dt, tag="wkT")
            for a in range(NDC):
                for b in range(NDC):
                    pswt = trp.tile([128, 128], tr_dt, tag="tr", name="pswt")
                    if MM_MODE == "f32r":
                        wsrc = wld[:, 0 * NDC + b, a * 128:(a + 1) * 128]
                    else:
                        wsrc = wk_sb[:, b, a * 128:(a + 1) * 128]
                    nc.tensor.transpose(pswt[:], wsrc, ident[:])
                    nc.vector.tensor_copy(wkT_sb[:, a, b * 128:(b + 1) * 128], pswt[:])

            # ---- phases 0+1 fused: load + transpose + project per chunk ----
            # X chunks feed V-projection (per chunk) and KT (per group of 4)
            for t in range(NSC):
                xld = ld.tile([128, D], F32, tag="ld")
                nc.sync.dma_start(xld[:], x_d[t * 128:(t + 1) * 128, :])
                if cast2b:
                    xldc = ld.tile([128, D], sb_dt, tag="ldc")
                    nc.vector.tensor_copy(xldc[:], xld[:])
                    xsrc = xldc
                else:
                    xsrc = xld
                for kc in range(NDC):
                    ps = trp.tile([128, 128], tr_dt, tag="tr")
                    nc.tensor.transpose(ps[:], xsrc[:, kc * 128:(kc + 1) * 128], ident[:])
                    if (t + kc) % 2 == 0:
                        nc.scalar.activation(xt[:, kc, t * 128:(t + 1) * 128], ps[:], Copy)
                    else:
                        nc.vector.tensor_copy(xt[:, kc, t * 128:(t + 1) * 128], ps[:])
                # V[t, :256] = X_t @ Wv + bv ; V[t, 256:] = 1
                psv = mmp.tile([128, D], F32, tag="proj", name="psv")
                for kc in range(NDC):
                    nc.tensor.matmul(
                        psv[:],
                        xt[:, kc, t * 128:(t + 1) * 128],
                        wv_sb[:, kc, :],
                        start=(kc == 0), stop=(kc == NDC - 1),
                    )
                nc.vector.tensor_add(vt[:, t, 0:D], psv[:], bv_bc[:])
            nc.vector.tensor_copy(vt[:, :, D:D + VPAD], vone_f32[:])

            # G chunks feed QT (per group of 4)
            for t in range(SQH // 128):
                gld = ld.tile([128, D], F32, tag="ld")
                nc.sync.dma_start(gld[:], g_d[t * 128:(t + 1) * 128, :])
                if cast2b:
                    gldc = ld.tile([128, D], sb_dt, tag="ldc")
                    nc.vector.tensor_copy(gldc[:], gld[:])
                    gsrc = gldc
                else:
                    gsrc = gld
                for kc in range(NDC):
                    ps = trp.tile([128, 128], tr_dt, tag="tr")
                    nc.tensor.transpose(ps[:], gsrc[:, kc * 128:(kc + 1) * 128], ident[:])
                    if (t + kc) % 2 == 0:
                        nc.scalar.activation(gt[:, kc, t * 128:(t + 1) * 128], ps[:], Copy)
                    else:
                        nc.vector.tensor_copy(gt[:, kc, t * 128:(t + 1) * 128], ps[:])
                if t % 4 == 3:
                    nt = t // 4
                    for dc in range(NDC):
                        psq = mmp.tile([128, 512], F32, tag="proj", name="psq")
                        for kc in range(NDC):
                            nc.tensor.matmul(
                                psq[:],
                                wq_sb[:, kc, dc * 128:(dc + 1) * 128],
                                gt[:, kc, nt * 512:(nt + 1) * 512],
                                start=(kc == 0), stop=(kc == NDC - 1),
                            )
                        nc.vector.tensor_scalar_add(
                            qt_sb[:, dc, nt * 512:(nt + 1) * 512], psq[:], bq_sb[:, dc, :])
                    # YT[dc, q] = sum_a Wk^T[a, dc-block] @ QhatT[a, q]
                    for dc in range(NDC):
                        psy = mmp.tile([128, 512], F32, tag="proj", name="psy")
                        for a in range(NDC):
                            nc.tensor.matmul(
                                psy[:],
                                wkT_sb[:, a, dc * 128:(dc + 1) * 128],
                                qt_sb[:, a, nt * 512:(nt + 1) * 512],
                                start=(a == 0), stop=(a == NDC - 1),
                            )
                        if (dc % 2) == 0:
                            nc.scalar.activation(
                                yt[:, dc, nt * 512:(nt + 1) * 512], psy[:], Copy)
                        else:
                            nc.vector.tensor_copy(
                                yt[:, dc, nt * 512:(nt + 1) * 512], psy[:])

        # ---- phase 2: attention ----
        es_bufs = 2 if cast2b else 1
        esp = ctx.enter_context(tc.tile_pool(name="esp", bufs=es_bufs))
        # each stp tile spans 2 PSUM banks so one ACTIVATE handles 2 kv-chunks
        stp = ctx.enter_context(tc.tile_pool(name="stp", bufs=2, space="PSUM"))
        pvp = ctx.enter_context(tc.tile_pool(name="pvp", bufs=1, space="PSUM"))
        osb_p = ctx.enter_context(tc.tile_pool(name="osb", bufs=4))

        inv_sqrt_d = 1.0 / float(np.sqrt(D))
        nqs = QT // 128
        HSC = NSC // 2
        if bench_reps:
            loop_cm = tc.For_i(0, bench_reps, 1)
        else:
            import contextlib
            loop_cm = contextlib.nullcontext()
        with loop_cm:
            emit_phase2(tc, nc, ctx, esp, stp, pvp, osb_p, xt, yt, vt, out_d,
                        sb_dt, inv_sqrt_d, nqs, HSC, Exp)

    nc.compile()
    return nc


def emit_phase2(tc, nc, ctx, esp, stp, pvp, osb_p, xt, yt, vt, out_d,
                sb_dt, inv_sqrt_d, nqs, HSC, Exp):
    if True:
        for qi in range(NQT):
            q0 = qi * QT
            # es in two halves: frees the first half's WAR dependency midway
            # through the PV pass so the next q-tile's exp can start earlier
            if ES_SPLIT:
                es_a = esp.tile([128, HSC, QT], sb_dt, tag="esa", name="es_a")
                es_b = esp.tile([128, HSC, QT], sb_dt, tag="esb", name="es_b")
                halves = (es_a, es_b)
            else:
                es = esp.tile([128, NSC, QT], sb_dt, tag="es", name="es")
                halves = (es[:, 0:HSC, :], es[:, HSC:NSC, :])
            for tp in range(NSC // 2):
                ps = stp.tile([128, 2 * QT], F32, tag="st")
                for sub in range(2):
                    t = 2 * tp + sub
                    for kc in range(NDC):
                        nc.tensor.matmul(
                            ps[:, sub * QT:(sub + 1) * QT],
                            xt[:, kc, t * 128:(t + 1) * 128],
                            yt[:, kc, q0:q0 + QT],
                            start=(kc == 0), stop=(kc == NDC - 1),
                        )
                eh = halves[(2 * tp) // HSC]
                nc.scalar.activation(eh[:, (2 * tp) % HSC:(2 * tp) % HSC + 2, :],
                                     ps[:], Exp, scale=inv_sqrt_d)
            accs = []
            for qs in range(nqs):
                acc_t = pvp.tile([128, D + VPAD], F32, tag=f"acc{qs}", name=f"acc{qs}")
                accs.append(acc_t)
            for t in range(NSC):
                eh = halves[t // HSC]
                for qs in range(nqs):
                    nc.tensor.matmul(
                        accs[qs][:],
                        eh[:, t % HSC, qs * 128:(qs + 1) * 128],
                        vt[:, t, :],
                        start=(t == 0), stop=(t == NSC - 1),
                    )
            for qs in range(nqs):
                acc = accs[qs]
                osb = osb_p.tile([128, D], F32, tag="osb")
                rec = osb_p.tile([128, 1], F32, tag="rec")
                nc.vector.reciprocal(rec[:], acc[:, D:D + 1])
                nc.vector.tensor_scalar_mul(osb[:], acc[:, 0:D], rec[:])
                nc.sync.dma_start(
                    out_d[q0 + qs * 128:q0 + (qs + 1) * 128, :], osb[:]
                )


def _get_program():
    if "nc" not in _CACHED:
        _CACHED["nc"] = build_program()
    return _CACHED["nc"]


def kernel(conv_local, conv_global, Wk, bk, Wq, bq, Wv, bv):
    nc = _get_program()
    conv_local = np.ascontiguousarray(np.asarray(conv_local, dtype=np.float32))
    conv_global = np.ascontiguousarray(np.asarray(conv_global, dtype=np.float32))
    wk = np.ascontiguousarray(np.asarray(Wk, dtype=np.float32))
    wq = np.ascontiguousarray(np.asarray(Wq, dtype=np.float32))
    wv = np.ascontiguousarray(np.asarray(Wv, dtype=np.float32))
    bq = np.ascontiguousarray(np.asarray(bq, dtype=np.float32).reshape(D, 1))
    bv = np.ascontiguousarray(np.asarray(bv, dtype=np.float32).reshape(1, D))

    in_maps = []
    for c in range(NCORES):
        b, h = c // 2, c % 2
        in_maps.append({
            "x": conv_local[b],
            "g": np.ascontiguousarray(conv_global[b, h * SQH:(h + 1) * SQH]),
            "wk": wk, "wq": wq, "wv": wv,
            "bq": bq, "bv": bv,
        })

    trace = bool(int(os.environ.get("ATTN_TRACE", "0")))
    res = run_bass_kernel_spmd(nc, in_maps, list(range(NCORES)), trace=trace)
    _CACHED["last_results"] = res

    out = np.empty((B, S, D), dtype=np.float32)
    for c in range(NCORES):
        b, h = c // 2, c % 2
        out[b, h * SQH:(h + 1) * SQH] = res.results[c]["out"]
    return out
